# revision 8
# baseline (speedup 1.0000x reference)
"""Trainium2 Bass kernel for nn_DomainQueue.

Math: out[b,l,c] = x_normed * sig_mix + mu_mix = x[b,l,c] * scale[b,c] + bias[b,c]
where
  mu/sig     = per-sample mean / unbiased std over L (computed on device)
  mu1/sig1   = queue gather: either a host-known queue row (no scatter hit) or
               the freshly computed stats of some other sample `src` (hit).
               Hit structure depends only on the integer inputs -> resolved on host.
  mu_mix     = lmda*mu + (1-lmda)*mu1   (same for sig_mix)
  scale      = sig_mix / sig ; bias = mu_mix - mu*scale
The final queue write (row -1) does not affect the returned tensor.

Sharding: channel-parallel over 8 cores (64 channels each). Each core computes
stats for ALL 64 samples over its channel slice, so the cross-sample gather is
fully local: no collectives, no redundant x reads.

Device layout per core: x packed on host to [128, 32*1024] where
partition p = (b%2)*64 + channel, free column = (b//2)*1024 + l.
So per-(b,c) stats are per-partition scalars -> bn_stats over the free dim and
a single per-partition-scalar multiply-add for the apply.
"""

import numpy as np

B, L, C = 64, 1024, 512
D = 5
CAPACITY = 1024
NUM_DOMAINS = 4
EPS = 1e-6
NCORES = 8
CS = C // NCORES          # 64 channels per core
NPAIR = B // 2            # 32 sample-pair columns
P = 128                   # partitions: (b%2)*CS + c
CH = 4                    # pairs per chunk (pipeline granularity)
NB = NPAIR // CH          # 8 chunks
FREE = NPAIR * L

_PERM = (3, 1, 4, 0, 2)   # involution: [t,i,l,k,c] <-> [k,i,c,t,l]

_NC_CACHE = {}
LAST_RESULT = None        # BassKernelResults of the most recent run (for test.py)


def _host_prepare(domain, mean_queue, sig_queue, d_ind1, f_ind1, offsets):
    """Resolve the scatter/gather index structure on the host.

    Returns (hits, Qmu, Qsig): hits is a list of (dst_b, src_b) pairs where the
    gather for sample dst_b must read the device-computed stats of src_b;
    Qmu/Qsig [B, C] hold the original queue rows for non-hit samples.
    """
    dom = np.asarray(domain).astype(np.int64).ravel()
    off = np.asarray(offsets).astype(np.int64).ravel()
    di = np.asarray(d_ind1).astype(np.int64).ravel()
    fi = np.asarray(f_ind1).astype(np.int64).ravel()
    mq = np.ascontiguousarray(np.asarray(mean_queue, dtype=np.float32))
    sq = np.ascontiguousarray(np.asarray(sig_queue, dtype=np.float32))

    counts = {}
    written = {}
    for b in range(B):
        d = int(dom[b])
        r = counts.get(d, 0)
        counts[d] = r + 1
        p = (int(off[d]) + r) % CAPACITY
        written[(d, p)] = b  # positions are unique per domain (B <= CAPACITY)

    hits = []
    Qmu = np.zeros((B, C), np.float32)
    Qsig = np.zeros((B, C), np.float32)
    for b in range(B):
        key = (int(di[b]), int(fi[b]))
        src = written.get(key)
        if src is not None:
            hits.append((b, src))
        else:
            Qmu[b] = mq[key[0], key[1]]
            Qsig[b] = sq[key[0], key[1]]
    return hits, Qmu, Qsig


def _build(hits):
    import concourse.bacc as bacc
    import concourse.mybir as mybir
    from concourse.tile import TileContext

    f32 = mybir.dt.float32
    ALU = mybir.AluOpType
    AF = mybir.ActivationFunctionType

    nc = bacc.Bacc("TRN2", target_bir_lowering=False, debug=False,
                   num_devices=NCORES, name="domain_queue")
    xin = nc.dram_tensor("xin", [P, FREE], f32, kind="ExternalInput")
    # aux = [qmu | qsig | lam] packed so ONE DMA (one semaphore) covers all
    # small inputs: Trn2 instructions only take 1-2 sync waits.
    aux_d = nc.dram_tensor("aux", [P, 3 * NPAIR], f32, kind="ExternalInput")
    yout = nc.dram_tensor("yout", [P, FREE], f32, kind="ExternalOutput")

    with TileContext(nc) as tc:
        with (
            tc.tile_pool(name="xp", bufs=NB) as xp,
            tc.tile_pool(name="stats", bufs=2 * NB) as stp,
            tc.tile_pool(name="small", bufs=1) as sp,
        ):
            eps_t = sp.tile([P, 1], f32, tag="eps")
            nc.vector.memset(eps_t[:], EPS)
            aux_t = sp.tile([P, 3 * NPAIR], f32, tag="aux")
            nc.sync.dma_start(out=aux_t[:], in_=aux_d[:])
            qm_all = aux_t[:, 0:NPAIR]
            qs_all = aux_t[:, NPAIR:2 * NPAIR]
            lam_t = aux_t[:, 2 * NPAIR:3 * NPAIR]
            # 1 - lam on device (DVE) so readers don't need a second DMA sem
            lam1_t = sp.tile([P, NPAIR], f32, tag="lam1")
            nc.vector.tensor_scalar(
                out=lam1_t[:], in0=lam_t, scalar1=-1.0, scalar2=1.0,
                op0=mybir.AluOpType.mult, op1=mybir.AluOpType.add,
            )

            chunks = []
            for i in range(NB):
                xt = xp.tile([P, CH * L], f32, tag="x")
                nc.sync.dma_start(out=xt[:], in_=xin[:, i * CH * L:(i + 1) * CH * L])
                chunks.append({"xt": xt})

            # per-(b,c) stats: bn_stats/bn_aggr give mean + population variance
            for ch in chunks:
                xt = ch["xt"]
                mv = stp.tile([P, CH, 2], f32, tag="mv")
                for j in range(CH):
                    st6 = stp.tile([P, 2, 6], f32, tag="st6")
                    for s in range(2):
                        nc.vector.bn_stats(
                            out=st6[:, s, :],
                            in_=xt[:, j * L + s * 512: j * L + (s + 1) * 512],
                        )
                    nc.vector.bn_aggr(out=mv[:, j, :], in_=st6[:])
                sig = stp.tile([P, CH], f32, tag="sig")
                # sig = sqrt(var_pop * L/(L-1) + eps)   (unbiased var, torch ddof=1)
                nc.scalar.activation(
                    out=sig[:], in_=mv[:, :, 1], func=AF.Sqrt,
                    bias=eps_t[:, 0:1], scale=float(L) / float(L - 1),
                )
                rsig = stp.tile([P, CH], f32, tag="rsig")
                nc.vector.reciprocal(out=rsig[:], in_=sig[:])
                ch["mv"] = mv
                ch["sig"] = sig
                ch["rsig"] = rsig

            # mix: mmix = lam*mu + (1-lam)*Q  (hit columns have Q=0, fixed below)
            for i, ch in enumerate(chunks):
                mu = ch["mv"][:, :, 0]
                cols = slice(i * CH, (i + 1) * CH)
                t1 = stp.tile([P, CH], f32, tag="t1")
                t2 = stp.tile([P, CH], f32, tag="t2")
                mmix = stp.tile([P, CH], f32, tag="mmix")
                nc.vector.tensor_mul(out=t1[:], in0=lam_t[:, cols], in1=mu)
                nc.vector.tensor_mul(out=t2[:], in0=lam1_t[:, cols], in1=qm_all[:, cols])
                nc.vector.tensor_add(out=mmix[:], in0=t1[:], in1=t2[:])
                smix = stp.tile([P, CH], f32, tag="smix")
                nc.vector.tensor_mul(out=t1[:], in0=lam_t[:, cols], in1=ch["sig"][:])
                nc.vector.tensor_mul(out=t2[:], in0=lam1_t[:, cols], in1=qs_all[:, cols])
                nc.vector.tensor_add(out=smix[:], in0=t1[:], in1=t2[:])
                ch["mmix"] = mmix
                ch["smix"] = smix

            # gather hits: mmix[b] += (1-lam_b) * mu[src] (partition-shifted
            # stat column fetched via a tiny sbuf->sbuf DMA; Q[b] was zeroed)
            for (b, src) in hits:
                ic, jc = divmod(b // 2, CH)
                isrc, jsrc = divmod(src // 2, CH)
                pd = (b % 2) * CS
                ps = (src % 2) * CS
                hm = stp.tile([P, 2], f32, tag="hm")
                nc.sync.dma_start(
                    out=hm[pd:pd + CS, 0:1],
                    in_=chunks[isrc]["mv"][ps:ps + CS, jsrc, 0:1],
                )
                nc.sync.dma_start(
                    out=hm[pd:pd + CS, 1:2],
                    in_=chunks[isrc]["sig"][ps:ps + CS, jsrc:jsrc + 1],
                )
                ht = stp.tile([P, 2], f32, tag="ht")
                nc.vector.tensor_scalar_mul(
                    out=ht[pd:pd + CS, :],
                    in0=hm[pd:pd + CS, :],
                    scalar1=lam1_t[pd:pd + CS, b // 2:b // 2 + 1],
                )
                nc.vector.tensor_add(
                    out=chunks[ic]["mmix"][pd:pd + CS, jc:jc + 1],
                    in0=chunks[ic]["mmix"][pd:pd + CS, jc:jc + 1],
                    in1=ht[pd:pd + CS, 0:1],
                )
                nc.vector.tensor_add(
                    out=chunks[ic]["smix"][pd:pd + CS, jc:jc + 1],
                    in0=chunks[ic]["smix"][pd:pd + CS, jc:jc + 1],
                    in1=ht[pd:pd + CS, 1:2],
                )

            # fold into per-(b,c) scale/bias
            for i, ch in enumerate(chunks):
                mu = ch["mv"][:, :, 0]
                scl = stp.tile([P, CH], f32, tag="scl")
                nc.vector.tensor_mul(out=scl[:], in0=ch["smix"][:], in1=ch["rsig"][:])
                bia = stp.tile([P, CH], f32, tag="bia")
                nc.vector.tensor_mul(out=bia[:], in0=mu, in1=scl[:])
                nc.vector.tensor_sub(out=bia[:], in0=ch["mmix"][:], in1=bia[:])
                ch["scl"] = scl
                ch["bia"] = bia

            # apply in place (ACT engine: out = x*scale + bias) and store
            for i, ch in enumerate(chunks):
                xt = ch["xt"]
                for j in range(CH):
                    xs = xt[:, j * L:(j + 1) * L]
                    nc.scalar.activation(
                        out=xs, in_=xs, func=AF.Identity,
                        bias=ch["bia"][:, j:j + 1], scale=ch["scl"][:, j:j + 1],
                    )
                nc.sync.dma_start(
                    out=yout[:, i * CH * L:(i + 1) * CH * L], in_=xt[:]
                )
    nc.compile()
    return nc


def _ensure_axon_hooks_module():
    """bass_utils imports antenv.axon_hooks when tracing is requested; some
    images lack that module. Provide a no-op shim so the import never fails."""
    import sys
    import types
    try:
        import antenv.axon_hooks  # noqa: F401
        return
    except ImportError:
        pass
    mod = types.ModuleType("antenv.axon_hooks")
    mod._hook = None
    mod.set_axon_ntff_profile_hook = lambda h: setattr(mod, "_hook", h)
    mod.get_axon_ntff_profile_hook = lambda: mod._hook
    sys.modules["antenv.axon_hooks"] = mod
    try:
        import antenv
        antenv.axon_hooks = mod
    except ImportError:
        pass


def kernel(x, domain, mean_queue, sig_queue, lmda, d_ind1, f_ind1, offsets):
    global LAST_RESULT
    _ensure_axon_hooks_module()
    from concourse.bass_utils import run_bass_kernel_spmd

    hits, Qmu, Qsig = _host_prepare(
        domain, mean_queue, sig_queue, d_ind1, f_ind1, offsets
    )

    key = tuple(sorted(hits))
    nc = _NC_CACHE.get(key)
    if nc is None:
        nc = _build(hits)
        _NC_CACHE[key] = nc

    # pack x: [B,L,C] -> per-core [P, FREE]
    xf = np.asarray(x, np.float32)
    xall = xf.reshape(NPAIR, 2, L, NCORES, CS).transpose(_PERM)  # [k,i,c,t,l]
    qmu_all = Qmu.reshape(NPAIR, 2, NCORES, CS).transpose(2, 1, 3, 0)   # [k,i,c,t]
    qsig_all = Qsig.reshape(NPAIR, 2, NCORES, CS).transpose(2, 1, 3, 0)
    lamv = np.asarray(lmda, np.float32).reshape(B)
    lam_t = np.repeat(lamv.reshape(NPAIR, 2).T, CS, axis=0)  # [P, NPAIR]

    in_maps = []
    for k in range(NCORES):
        aux = np.empty((P, 3 * NPAIR), np.float32)
        aux[:, 0:NPAIR] = qmu_all[k].reshape(P, NPAIR)
        aux[:, NPAIR:2 * NPAIR] = qsig_all[k].reshape(P, NPAIR)
        aux[:, 2 * NPAIR:3 * NPAIR] = lam_t
        in_maps.append({
            "xin": np.ascontiguousarray(xall[k]).reshape(P, FREE),
            "aux": aux,
        })

    res = run_bass_kernel_spmd(nc, in_maps, core_ids=list(range(NCORES)))
    LAST_RESULT = res

    ys = np.stack([res.results[k]["yout"].reshape(2, CS, NPAIR, L)
                   for k in range(NCORES)])          # [k,i,c,t,l]
    y = ys.transpose(_PERM).reshape(B, L, C)         # [t,i,l,k,c] -> [B,L,C]
    return np.ascontiguousarray(y)
